# revision 11
# baseline (speedup 1.0000x reference)
"""CrossViewAttention Trainium2 kernel.

Math: for each batch row b with features f1, f2 (D=1024):
  Q_s = f_s Wq^T + bq ; K_t = f_t Wk^T + bk ; V_t = f_t Wv^T + bv
  scores s_st = Q_s.K_t / sqrt(D); attn = softmax over t; out = sum_s attn_st V_t

2-way softmax collapses to sigmoids of score differences:
  d1 = (s11-s12) = (f1.(g @ M^T) + g.ck)/sqrt(D)
  d2 = (s21-s22) = (f2.(g @ M^T) + g.ck)/sqrt(D)
  with g = f1-f2, M = Wq^T Wk, ck = Wk^T bq  (bk and bq-cross terms cancel)
  w1 = sigmoid(d1)+sigmoid(d2); w2 = 2-w1
  out = (w1*f1 + w2*f2) @ Wv^T + 2*bv

So per 128-row chunk only TWO 128x1024x1024 matmuls are needed (scores + output),
run in fp32r (tf32-like) on the PE at 1 cycle/row.  The rank-1 g.ck correction
and M^T itself are precomputed on the host; g.ck/sqrt(D) seeds the fused
multiply-reduce as its initial value.

Sharding: batch split across 8 cores (2048 rows each), weights replicated.
"""

import sys

for _p in ("/opt/trn_rl_repo",):
    if _p not in sys.path:
        sys.path.insert(0, _p)

import numpy as np

import concourse.bacc as bacc
import concourse.mybir as mybir
import concourse.tile as tile

F32 = mybir.dt.float32
F32R = mybir.dt.float32r

B = 16384
D = 1024
NCORES = 8
R = B // NCORES          # rows per core
CH = 128                 # chunk rows
KT = D // 128            # contraction k-tiles (8)
SCALE = 1.0 / float(np.sqrt(D))


def build(nc, n_chunks, repeats=1):
    f1s = nc.dram_tensor("f1s", [n_chunks * CH, D], F32, kind="ExternalInput").ap()
    f2s = nc.dram_tensor("f2s", [n_chunks * CH, D], F32, kind="ExternalInput").ap()
    gtb = nc.dram_tensor("gtb", [n_chunks, KT, 128, CH], F32R, kind="ExternalInput").ap()
    gckb = nc.dram_tensor("gckb", [128, n_chunks], F32, kind="ExternalInput").ap()
    mtb = nc.dram_tensor("mtb", [KT, 128, D], F32R, kind="ExternalInput").ap()
    wvt = nc.dram_tensor("wvt", [KT, 128, D], F32R, kind="ExternalInput").ap()
    bv2f = nc.dram_tensor("bv2f", [128, D], F32, kind="ExternalInput").ap()
    idn = nc.dram_tensor("idn", [128, 128], F32R, kind="ExternalInput").ap()
    out = nc.dram_tensor("out", [n_chunks * CH, D], F32, kind="ExternalOutput").ap()

    with tile.TileContext(nc) as tc:
        with (
            tc.tile_pool(name="wpool", bufs=1) as wpool,
            tc.tile_pool(name="io", bufs=3) as io,
            tc.tile_pool(name="work", bufs=2) as work,
            tc.tile_pool(name="small", bufs=2) as small,
            tc.tile_pool(name="ps_ud", bufs=2, space="PSUM") as ps_ud,
            tc.tile_pool(name="ps_xt", bufs=1, space="PSUM") as ps_xt,
            tc.tile_pool(name="ps_o", bufs=1, space="PSUM") as ps_o,
        ):
            # resident weights
            mt_sb = wpool.tile([128, KT * D], F32R)
            for k in range(KT):
                nc.sync.dma_start(mt_sb[:, k * D : (k + 1) * D], mtb[k, :, :])
            wv_sb = wpool.tile([128, KT * D], F32R)
            for k in range(KT):
                nc.sync.dma_start(wv_sb[:, k * D : (k + 1) * D], wvt[k, :, :])
            bv_sb = wpool.tile([128, D], F32)
            nc.sync.dma_start(bv_sb[:], bv2f[:])
            id_sb = wpool.tile([128, 128], F32R)
            nc.sync.dma_start(id_sb[:], idn[:])
            gck_sb = wpool.tile([128, n_chunks], F32)
            nc.sync.dma_start(gck_sb[:], gckb[:])

            for i in range(n_chunks * repeats):
                i = i % n_chunks
                rs = i * CH
                # ---- loads
                f1t = io.tile([128, D], F32, tag="f1t")
                nc.sync.dma_start(f1t[:], f1s[rs : rs + CH, :])
                f2t = io.tile([128, D], F32, tag="f2t")
                nc.sync.dma_start(f2t[:], f2s[rs : rs + CH, :])
                gt = io.tile([128, D], F32R, tag="gt")
                for k in range(KT):
                    nc.sync.dma_start(
                        gt[:, k * 128 : (k + 1) * 128], gtb[i, k, :, :]
                    )

                # ---- mm1: Ud = g @ M^T   -> psum [128, 1024]
                ud = ps_ud.tile([128, D], F32, tag="ud")
                for k in range(KT):
                    lhs = gt[:, k * 128 : (k + 1) * 128]
                    st = k == 0
                    sp = k == KT - 1
                    nc.tensor.matmul(
                        ud[:, 0:512],
                        lhs,
                        mt_sb[:, k * D : k * D + 512],
                        start=st,
                        stop=sp,
                    )
                    nc.tensor.matmul(
                        ud[:, 512:1024],
                        lhs,
                        mt_sb[:, k * D + 512 : k * D + 1024],
                        start=st,
                        stop=sp,
                    )

                # ---- dots: d_s = sum(f_s * Ud)/sqrt(D)   (g.ck/sqrt(D) added
                #      later as the sigmoid's per-partition bias)
                dd = small.tile([128, 2], F32, tag="dd")
                scr1 = work.tile([128, D], F32, tag="scr")
                nc.vector.scalar_tensor_tensor(
                    out=scr1[:],
                    in0=f1t[:],
                    scalar=SCALE,
                    in1=ud[:],
                    op0=mybir.AluOpType.mult,
                    op1=mybir.AluOpType.mult,
                    accum_out=dd[:, 0:1],
                )
                scr2 = work.tile([128, D], F32, tag="scr")
                nc.vector.scalar_tensor_tensor(
                    out=scr2[:],
                    in0=f2t[:],
                    scalar=SCALE,
                    in1=ud[:],
                    op0=mybir.AluOpType.mult,
                    op1=mybir.AluOpType.mult,
                    accum_out=dd[:, 1:2],
                )

                # ---- w1 = sig(d1 + gck)+sig(d2 + gck), w2 = 2-w1
                sg = small.tile([128, 2], F32, tag="sg")
                nc.scalar.activation(
                    sg[:],
                    dd[:],
                    mybir.ActivationFunctionType.Sigmoid,
                    bias=gck_sb[:, i : i + 1],
                )
                w1 = small.tile([128, 1], F32, tag="w1")
                nc.vector.tensor_tensor(
                    w1[:], sg[:, 0:1], sg[:, 1:2], op=mybir.AluOpType.add
                )
                w2 = small.tile([128, 1], F32, tag="w2")
                nc.vector.tensor_scalar(
                    w2[:],
                    w1[:],
                    -1.0,
                    2.0,
                    op0=mybir.AluOpType.mult,
                    op1=mybir.AluOpType.add,
                )

                # ---- X = w1*f1 + w2*f2   (f32r)
                t1 = work.tile([128, D], F32, tag="t1")
                nc.vector.tensor_scalar(
                    t1[:], f2t[:], w2[:], None, op0=mybir.AluOpType.mult
                )
                xr = work.tile([128, D], F32R, tag="xr")
                nc.vector.scalar_tensor_tensor(
                    out=xr[:],
                    in0=f1t[:],
                    scalar=w1[:],
                    in1=t1[:],
                    op0=mybir.AluOpType.mult,
                    op1=mybir.AluOpType.add,
                )

                # ---- X^T via PE transpose (per 128-block), psum f32r
                xt_ps = ps_xt.tile([128, D], F32R, tag="xt")
                for k in range(KT):
                    nc.tensor.transpose(
                        xt_ps[:, k * 128 : (k + 1) * 128],
                        xr[:, k * 128 : (k + 1) * 128],
                        id_sb[:],
                    )
                xt = work.tile([128, D], F32R, tag="xts")
                nc.scalar.copy(xt[:], xt_ps[:])

                # ---- mm2: out = X @ Wv^T  -> psum [128, 1024]
                po = ps_o.tile([128, D], F32, tag="po")
                for k in range(KT):
                    lhs = xt[:, k * 128 : (k + 1) * 128]
                    st = k == 0
                    sp = k == KT - 1
                    nc.tensor.matmul(
                        po[:, 0:512],
                        lhs,
                        wv_sb[:, k * D : k * D + 512],
                        start=st,
                        stop=sp,
                    )
                    nc.tensor.matmul(
                        po[:, 512:1024],
                        lhs,
                        wv_sb[:, k * D + 512 : k * D + 1024],
                        start=st,
                        stop=sp,
                    )

                # ---- += 2bv (broadcast tile) and store
                ob = work.tile([128, D], F32, tag="ob")
                nc.vector.tensor_tensor(
                    ob[:], po[:], bv_sb[:], op=mybir.AluOpType.add
                )
                nc.sync.dma_start(out[rs : rs + CH, :], ob[:])

    return out


_CACHE = {}


def get_compiled(n_chunks=R // CH):
    key = n_chunks
    if key not in _CACHE:
        nc = bacc.Bacc(
            "TRN2", target_bir_lowering=False, debug=False, num_devices=NCORES
        )
        build(nc, n_chunks)
        nc.compile()
        _CACHE[key] = nc
    return _CACHE[key]


def prep_inputs(f1, f2, Wq, bq, Wk, bk, Wv, bv):
    """Host-side algebra + sharding. Returns per-core input maps."""
    f1 = np.ascontiguousarray(np.asarray(f1), dtype=np.float32)
    f2 = np.ascontiguousarray(np.asarray(f2), dtype=np.float32)
    Wq = np.asarray(Wq, dtype=np.float32)
    bq = np.asarray(bq, dtype=np.float32)
    Wk = np.asarray(Wk, dtype=np.float32)
    Wv = np.asarray(Wv, dtype=np.float32)
    bv = np.asarray(bv, dtype=np.float32)
    g = f1 - f2

    WkT = np.ascontiguousarray(Wk.T)
    MT = WkT @ Wq                             # M^T = Wk^T Wq  [D, D]
    ck = WkT @ bq                             # [D]
    gck = (g @ ck) * np.float32(SCALE)        # [B]
    mtb = np.ascontiguousarray(MT.reshape(KT, 128, D))
    wvt = np.ascontiguousarray(Wv.T).reshape(KT, 128, D)
    bv2f = np.broadcast_to(2.0 * bv, (128, D)).astype(np.float32).copy()
    idn = np.eye(128, dtype=np.float32)

    n_chunks = R // CH
    in_maps = []
    for c in range(NCORES):
        sl = slice(c * R, (c + 1) * R)
        gs = g[sl]
        gtb = np.ascontiguousarray(
            gs.reshape(n_chunks, CH, KT, 128).transpose(0, 2, 3, 1)
        )
        gckb = np.ascontiguousarray(gck[sl].reshape(n_chunks, CH).T)
        in_maps.append(
            {
                "f1s": np.ascontiguousarray(f1[sl]),
                "f2s": np.ascontiguousarray(f2[sl]),
                "gtb": gtb,
                "gckb": gckb,
                "mtb": mtb,
                "wvt": wvt,
                "bv2f": bv2f,
                "idn": idn,
            }
        )
    return in_maps


def kernel(**inputs):
    from concourse.bass_utils import run_bass_kernel_spmd

    nc = get_compiled()
    in_maps = prep_inputs(**inputs)
    res = run_bass_kernel_spmd(nc, in_maps, core_ids=list(range(NCORES)))
    return np.concatenate([res.results[c]["out"] for c in range(NCORES)], axis=0)


# revision 12
# speedup vs baseline: 1.2471x; 1.2471x over previous
"""CrossViewAttention Trainium2 kernel.

Math: for each batch row b with features f1, f2 (D=1024):
  Q_s = f_s Wq^T + bq ; K_t = f_t Wk^T + bk ; V_t = f_t Wv^T + bv
  scores s_st = Q_s.K_t / sqrt(D); attn = softmax over t; out = sum_s attn_st V_t

2-way softmax collapses to sigmoids of score differences:
  d1 = (s11-s12) = (f1.(g @ M^T) + g.ck)/sqrt(D)
  d2 = (s21-s22) = (f2.(g @ M^T) + g.ck)/sqrt(D)
  with g = f1-f2, M = Wq^T Wk, ck = Wk^T bq  (bk and bq-cross terms cancel)
  w1 = sigmoid(d1)+sigmoid(d2); w2 = 2-w1
  out = (w1*f1 + w2*f2) @ Wv^T + 2*bv

So per 128-row chunk only TWO 128x1024x1024 matmuls are needed (scores + output),
run in fp32r (tf32-like) on the PE at 1 cycle/row.  The rank-1 g.ck correction
and M^T itself are precomputed on the host; g.ck/sqrt(D) seeds the fused
multiply-reduce as its initial value.

Sharding: batch split across 8 cores (2048 rows each), weights replicated.
"""

import sys

for _p in ("/opt/trn_rl_repo",):
    if _p not in sys.path:
        sys.path.insert(0, _p)

import numpy as np

import concourse.bacc as bacc
import concourse.mybir as mybir
import concourse.tile as tile

F32 = mybir.dt.float32
F32R = mybir.dt.float32r
BF16 = mybir.dt.bfloat16

B = 16384
D = 1024
NCORES = 8
R = B // NCORES          # rows per core
CH = 128                 # chunk rows
KT = D // 128            # contraction k-tiles (8)
SCALE = 1.0 / float(np.sqrt(D))


def build(nc, n_chunks, repeats=1):
    f1s = nc.dram_tensor("f1s", [n_chunks * CH, D], F32, kind="ExternalInput").ap()
    f2s = nc.dram_tensor("f2s", [n_chunks * CH, D], F32, kind="ExternalInput").ap()
    gtb = nc.dram_tensor("gtb", [n_chunks, KT, 128, CH], BF16, kind="ExternalInput").ap()
    gckb = nc.dram_tensor("gckb", [128, n_chunks], F32, kind="ExternalInput").ap()
    mtb = nc.dram_tensor("mtb", [KT, 128, D], BF16, kind="ExternalInput").ap()
    wvt = nc.dram_tensor("wvt", [KT, 128, D], F32R, kind="ExternalInput").ap()
    bv2f = nc.dram_tensor("bv2f", [128, D], F32, kind="ExternalInput").ap()
    idn = nc.dram_tensor("idn", [128, 128], F32R, kind="ExternalInput").ap()
    out = nc.dram_tensor("out", [n_chunks * CH, D], F32, kind="ExternalOutput").ap()

    with tile.TileContext(nc) as tc:
        with (
            tc.tile_pool(name="wpool", bufs=1) as wpool,
            tc.tile_pool(name="io", bufs=3) as io,
            tc.tile_pool(name="work", bufs=2) as work,
            tc.tile_pool(name="small", bufs=2) as small,
            tc.tile_pool(name="ps_ud", bufs=1, space="PSUM") as ps_ud,
            tc.tile_pool(name="ps_xt", bufs=1, space="PSUM") as ps_xt,
            tc.tile_pool(name="ps_o", bufs=1, space="PSUM") as ps_o,
        ):
            # resident weights
            mt_sb = wpool.tile([128, KT * D], BF16)
            for k in range(KT):
                nc.sync.dma_start(mt_sb[:, k * D : (k + 1) * D], mtb[k, :, :])
            wv_sb = wpool.tile([128, KT * D], F32R)
            for k in range(KT):
                nc.sync.dma_start(wv_sb[:, k * D : (k + 1) * D], wvt[k, :, :])
            bv_sb = wpool.tile([128, D], F32)
            nc.sync.dma_start(bv_sb[:], bv2f[:])
            id_sb = wpool.tile([128, 128], F32R)
            nc.sync.dma_start(id_sb[:], idn[:])
            gck_sb = wpool.tile([128, n_chunks], F32)
            nc.sync.dma_start(gck_sb[:], gckb[:])

            for i in range(n_chunks * repeats):
                i = i % n_chunks
                rs = i * CH
                # ---- loads
                f1t = io.tile([128, D], F32, tag="f1t")
                nc.sync.dma_start(f1t[:], f1s[rs : rs + CH, :])
                f2t = io.tile([128, D], F32, tag="f2t")
                nc.sync.dma_start(f2t[:], f2s[rs : rs + CH, :])
                gt = io.tile([128, D], BF16, tag="gt")
                for k in range(KT):
                    nc.sync.dma_start(
                        gt[:, k * 128 : (k + 1) * 128], gtb[i, k, :, :]
                    )

                # ---- mm1: Ud = g @ M^T   -> psum [128, 1024]
                ud = ps_ud.tile([128, D], F32, tag="ud")
                for k in range(KT):
                    lhs = gt[:, k * 128 : (k + 1) * 128]
                    st = k == 0
                    sp = k == KT - 1
                    nc.tensor.matmul(
                        ud[:, 0:512],
                        lhs,
                        mt_sb[:, k * D : k * D + 512],
                        start=st,
                        stop=sp,
                    )
                    nc.tensor.matmul(
                        ud[:, 512:1024],
                        lhs,
                        mt_sb[:, k * D + 512 : k * D + 1024],
                        start=st,
                        stop=sp,
                    )

                # ---- dots: d_s = sum(f_s * Ud)/sqrt(D)   (g.ck/sqrt(D) added
                #      later as the sigmoid's per-partition bias)
                dd = small.tile([128, 2], F32, tag="dd")
                scr1 = work.tile([128, D], F32, tag="scr")
                nc.vector.scalar_tensor_tensor(
                    out=scr1[:],
                    in0=f1t[:],
                    scalar=SCALE,
                    in1=ud[:],
                    op0=mybir.AluOpType.mult,
                    op1=mybir.AluOpType.mult,
                    accum_out=dd[:, 0:1],
                )
                scr2 = work.tile([128, D], F32, tag="scr")
                nc.vector.scalar_tensor_tensor(
                    out=scr2[:],
                    in0=f2t[:],
                    scalar=SCALE,
                    in1=ud[:],
                    op0=mybir.AluOpType.mult,
                    op1=mybir.AluOpType.mult,
                    accum_out=dd[:, 1:2],
                )

                # ---- w1 = sig(d1 + gck)+sig(d2 + gck), w2 = 2-w1
                sg = small.tile([128, 2], F32, tag="sg")
                nc.scalar.activation(
                    sg[:],
                    dd[:],
                    mybir.ActivationFunctionType.Sigmoid,
                    bias=gck_sb[:, i : i + 1],
                )
                w1 = small.tile([128, 1], F32, tag="w1")
                nc.vector.tensor_tensor(
                    w1[:], sg[:, 0:1], sg[:, 1:2], op=mybir.AluOpType.add
                )
                w2 = small.tile([128, 1], F32, tag="w2")
                nc.vector.tensor_scalar(
                    w2[:],
                    w1[:],
                    -1.0,
                    2.0,
                    op0=mybir.AluOpType.mult,
                    op1=mybir.AluOpType.add,
                )

                # ---- X = w1*f1 + w2*f2   (f32r)
                t1 = work.tile([128, D], F32, tag="t1")
                nc.vector.tensor_scalar(
                    t1[:], f2t[:], w2[:], None, op0=mybir.AluOpType.mult
                )
                xr = work.tile([128, D], F32R, tag="xr")
                nc.vector.scalar_tensor_tensor(
                    out=xr[:],
                    in0=f1t[:],
                    scalar=w1[:],
                    in1=t1[:],
                    op0=mybir.AluOpType.mult,
                    op1=mybir.AluOpType.add,
                )

                # ---- X^T via PE transpose (per 128-block), psum f32r
                xt_ps = ps_xt.tile([128, D], F32R, tag="xt")
                for k in range(KT):
                    nc.tensor.transpose(
                        xt_ps[:, k * 128 : (k + 1) * 128],
                        xr[:, k * 128 : (k + 1) * 128],
                        id_sb[:],
                    )
                xt = work.tile([128, D], F32R, tag="xts")
                nc.scalar.copy(xt[:], xt_ps[:])

                # ---- mm2: out = X @ Wv^T  -> psum [128, 1024]
                po = ps_o.tile([128, D], F32, tag="po")
                for k in range(KT):
                    lhs = xt[:, k * 128 : (k + 1) * 128]
                    st = k == 0
                    sp = k == KT - 1
                    nc.tensor.matmul(
                        po[:, 0:512],
                        lhs,
                        wv_sb[:, k * D : k * D + 512],
                        start=st,
                        stop=sp,
                    )
                    nc.tensor.matmul(
                        po[:, 512:1024],
                        lhs,
                        wv_sb[:, k * D + 512 : k * D + 1024],
                        start=st,
                        stop=sp,
                    )

                # ---- += 2bv (broadcast tile) and store
                ob = work.tile([128, D], F32, tag="ob")
                nc.vector.tensor_tensor(
                    ob[:], po[:], bv_sb[:], op=mybir.AluOpType.add
                )
                nc.sync.dma_start(out[rs : rs + CH, :], ob[:])

    return out


_CACHE = {}


def get_compiled(n_chunks=R // CH):
    key = n_chunks
    if key not in _CACHE:
        nc = bacc.Bacc(
            "TRN2", target_bir_lowering=False, debug=False, num_devices=NCORES
        )
        build(nc, n_chunks)
        nc.compile()
        _CACHE[key] = nc
    return _CACHE[key]


def prep_inputs(f1, f2, Wq, bq, Wk, bk, Wv, bv):
    """Host-side algebra + sharding. Returns per-core input maps."""
    f1 = np.ascontiguousarray(np.asarray(f1), dtype=np.float32)
    f2 = np.ascontiguousarray(np.asarray(f2), dtype=np.float32)
    Wq = np.asarray(Wq, dtype=np.float32)
    bq = np.asarray(bq, dtype=np.float32)
    Wk = np.asarray(Wk, dtype=np.float32)
    Wv = np.asarray(Wv, dtype=np.float32)
    bv = np.asarray(bv, dtype=np.float32)
    g = f1 - f2

    WkT = np.ascontiguousarray(Wk.T)
    MT = WkT @ Wq                             # M^T = Wk^T Wq  [D, D]
    ck = WkT @ bq                             # [D]
    gck = (g @ ck) * np.float32(SCALE)        # [B]
    import ml_dtypes
    mtb = np.ascontiguousarray(MT.reshape(KT, 128, D)).astype(ml_dtypes.bfloat16)
    wvt = np.ascontiguousarray(Wv.T).reshape(KT, 128, D)
    bv2f = np.broadcast_to(2.0 * bv, (128, D)).astype(np.float32).copy()
    idn = np.eye(128, dtype=np.float32)

    n_chunks = R // CH
    in_maps = []
    for c in range(NCORES):
        sl = slice(c * R, (c + 1) * R)
        gs = g[sl]
        gtb = np.ascontiguousarray(
            gs.reshape(n_chunks, CH, KT, 128).transpose(0, 2, 3, 1)
        ).astype(ml_dtypes.bfloat16)
        gckb = np.ascontiguousarray(gck[sl].reshape(n_chunks, CH).T)
        in_maps.append(
            {
                "f1s": np.ascontiguousarray(f1[sl]),
                "f2s": np.ascontiguousarray(f2[sl]),
                "gtb": gtb,
                "gckb": gckb,
                "mtb": mtb,
                "wvt": wvt,
                "bv2f": bv2f,
                "idn": idn,
            }
        )
    return in_maps


def kernel(**inputs):
    from concourse.bass_utils import run_bass_kernel_spmd

    nc = get_compiled()
    in_maps = prep_inputs(**inputs)
    res = run_bass_kernel_spmd(nc, in_maps, core_ids=list(range(NCORES)))
    return np.concatenate([res.results[c]["out"] for c in range(NCORES)], axis=0)
